# revision 10
# baseline (speedup 1.0000x reference)
"""Multi-head attention (B=2, T=2048, C=2048, H=16, causal, rotary) on 8
Trainium2 NeuronCores.

Sharding: tensor-parallel over heads x data-parallel over batch.
Core c handles batch b = c // 4 and heads [4*(c%4), 4*(c%4)+4).
Each core computes a partial output y_c = attn_out(4 heads) @ wo_rows;
the host sums the 4 partials per batch (row-parallel wo).

Fused single-phase design: Q^T/K^T/V are computed once and stay
SBUF-resident in bf16 (no DRAM spill round-trip).  All matmuls run in
bf16 (1 cycle/row, same PE rate as f32r); PSUM accumulation is fp32.

  stage A (per t-chunk 512): Q^T,K^T [d,t] per head via stationary-
          weight matmuls over host-pre-transposed x^T; RoPE applied in
          de-interleaved form (host permutes wq/wk columns so rows
          0:64 = real pairs, 64:128 = imag pairs) with stacked cos|sin
          tiles (2 muls + sub + add on DVE); V in natural [t,d] via
          gpsimd copies.  Results land in resident bf16 tiles.
  stage B (per q-chunk 512, head): S^T[k,q] tiles by one matmul each
          (contraction d=128), ACT exp with 1/sqrt(D) folded in, causal
          via block skip + staircase mask multiply on gpsimd, O^T
          accumulated with V stationary / E moving, row-sums via
          ones-column matmul, reciprocal broadcast via 1-wide matmul,
          normalize on DVE (deferred one iteration so the PE never
          waits on the scalar-engine copy latency).
  stage C: wo applied per q-chunk as soon as its 4 heads are
          normalized, with per-[128,512] y pieces DMA'd out on the
          sync queue so the output stream overlaps attention instead
          of draining at the end.
"""

import math
import os
import sys
from contextlib import ExitStack

import numpy as np

for _p in ("/opt/trn_rl_repo", "/root/.axon_site/_ro/trn_rl_repo"):
    if os.path.isdir(_p) and _p not in sys.path:
        sys.path.append(_p)

import bass_rust
import concourse.bass as bass
import concourse.mybir as mybir
import concourse.tile as tile
from concourse.bass_utils import run_bass_kernel_spmd
from concourse.vector_clock import ScopedClock, VectorClock

B, T, C, H = 2, 2048, 2048, 16
D = 128
HPC = H // 4          # 4 heads per core
DH = HPC * D          # 512 head-dims per core
NCH = C // 128        # 16 contraction chunks
TCH = 512             # stage-A t-chunk
QCH = 512             # stage-B q-chunk
NT = T // TCH
NQ = T // QCH
N_CORES = 8
SCALE = 1.0 / math.sqrt(D)

f32 = mybir.dt.float32
f32r = mybir.dt.float32r
bf16 = mybir.dt.bfloat16
AF = mybir.ActivationFunctionType


# --------------------------------------------------------------------------
# toolchain workarounds
# --------------------------------------------------------------------------
def _patched_drain_and_barrier(self, tick_clock, wait_clock):
    """walrus codegen accepts only one sem wait on an InstDrain; emit one
    drain per outstanding proc instead of one drain with N waits."""
    ticks = list(tick_clock.global_clock)
    for i, t in enumerate(ticks):
        if t <= 0:
            continue
        sub = VectorClock([t if j == i else 0 for j in range(len(ticks))])
        d = self.nc.sync.drain()
        wait_clock.add_sem_waits(d.ins, ScopedClock({None: sub}))
    self.nc.all_engine_barrier()
    assert self.sems is not None
    popped = self.nc._tile_sem_poison_stack.pop()
    assert popped is self._sem_poison
    self.nc.clear_and_free_semaphores(list(self.sems.allocated().values()))
    self.nc.all_engine_barrier()


tile.TileContext._drain_and_barrier = _patched_drain_and_barrier

_SPLIT_OPS = {
    "Matmult", "Drain", "DMACopy", "DMATransposeAnt", "Activation", "TensorTensor", "TensorReduce",
    "TensorCopy", "Reciprocal", "TensorScalarPtr", "TensorScalar",
    "CopyPredicated", "Memset", "NoOp", "Pool", "Max", "MaxIndex",
    "StreamShuffle", "StreamTranspose", "TensorTensorScan",
    "ScalarTensorTensor", "TensorTensorReduce", "Iota", "BNStats",
    "BNStatsAggregate", "Select",
}
_ws_counter = [0]


def _split_waits(nc, limit=1):
    """walrus encodes a limited number of sem waits on engine instructions
    (fused LDW+MM and Drain take only one). Move excess waits onto
    same-engine NoOps inserted immediately before; engine program order
    preserves semantics."""
    for f in nc.m.functions:
        for b in f.blocks:
            insts = b.instructions
            i = 0
            while i < len(insts):
                inst = insts[i]
                si = inst.sync_info
                if (
                    inst.opcode not in _SPLIT_OPS
                    or si is None
                    or not si.on_wait
                    or len(si.on_wait) <= limit
                ):
                    i += 1
                    continue
                waits = list(si.on_wait)
                extra, keep = waits[:-limit], waits[-limit:]
                for w in extra:
                    _ws_counter[0] += 1
                    nop = bass_rust.InstNoOp(
                        name=f"I-waitsplit-{_ws_counter[0]}", engine=inst.engine
                    )
                    nop.sync_info = mybir.SyncInfo(on_wait=[w], on_update=[])
                    insts.insert(i, nop)
                    i += 1
                inst.sync_info = mybir.SyncInfo(
                    on_wait=keep,
                    on_update=list(si.on_update) if si.on_update else [],
                )
                i += 1


# --------------------------------------------------------------------------
# kernel build
# --------------------------------------------------------------------------
def _build_nc():
    nc = bass.Bass("TRN2", debug=False, target_bir_lowering=False)

    xT = nc.dram_tensor("xT", [C, T], bf16, kind="ExternalInput").ap()
    wq = nc.dram_tensor("wq", [C, DH], bf16, kind="ExternalInput").ap()
    wk = nc.dram_tensor("wk", [C, DH], bf16, kind="ExternalInput").ap()
    wv = nc.dram_tensor("wv", [C, DH], bf16, kind="ExternalInput").ap()
    wo = nc.dram_tensor("wo", [DH, C], bf16, kind="ExternalInput").ap()
    cosT = nc.dram_tensor("cosT", [64, T], f32, kind="ExternalInput").ap()
    sinT = nc.dram_tensor("sinT", [64, T], f32, kind="ExternalInput").ap()
    mbd = nc.dram_tensor("mb", [128, 128], bf16, kind="ExternalInput").ap()
    onesk_d = nc.dram_tensor("onesk", [128, 1], bf16, kind="ExternalInput").ap()
    ones1_d = nc.dram_tensor("ones1", [1, 128], f32r, kind="ExternalInput").ap()
    y = nc.dram_tensor("y", [T, C], f32, kind="ExternalOutput").ap()

    with tile.TileContext(nc) as tc, ExitStack() as top:
        # resident pools (live for the whole kernel)
        wpool = top.enter_context(tc.tile_pool(name="w", bufs=1))
        wopool = top.enter_context(tc.tile_pool(name="wo", bufs=1))
        cspool = top.enter_context(tc.tile_pool(name="cs", bufs=1))
        cstp = top.enter_context(tc.tile_pool(name="cst", bufs=1))
        xpool = top.enter_context(tc.tile_pool(name="x", bufs=2))
        resq = top.enter_context(tc.tile_pool(name="qkv", bufs=1))
        rt = top.enter_context(tc.tile_pool(name="rt", bufs=2))
        ep = top.enter_context(tc.tile_pool(name="e", bufs=4))
        otp = top.enter_context(tc.tile_pool(name="ot", bufs=2))
        rp = top.enter_context(tc.tile_pool(name="r", bufs=2))
        bp = top.enter_context(tc.tile_pool(name="bsb", bufs=2))
        ysbp = top.enter_context(tc.tile_pool(name="ysb", bufs=4))

        # ---- weight/constant loads.  Queue order matters: each queue
        # executes its DMAs in program order and HBM bandwidth saturates
        # during the first t-chunk, so operands are queued in the order
        # the PE consumes them (wq -> cos/sin -> wk -> wv -> wo/consts).
        w_tiles = {}
        for wname in ("wq", "wk", "wv"):
            for ci in range(NCH):
                wt = wpool.tile([128, DH], bf16, tag=f"{wname}{ci}",
                                name=f"{wname}{ci}")
                w_tiles[(wname, ci)] = wt
        wdr = {"wq": wq, "wk": wk, "wv": wv}

        def _load_w(wname, parity, eng):
            for ci in range(parity, NCH, 2):
                eng.dma_start(
                    w_tiles[(wname, ci)][:], wdr[wname][ci * 128:(ci + 1) * 128, :]
                )

        _load_w("wq", 0, nc.sync)
        _load_w("wq", 1, nc.gpsimd)
        cos_t = cspool.tile([64, T], f32, tag="cos")
        nc.gpsimd.dma_start(cos_t[:], cosT)
        sin_t = cspool.tile([64, T], f32, tag="sin")
        nc.gpsimd.dma_start(sin_t[:], sinT)
        _load_w("wk", 0, nc.sync)
        _load_w("wk", 1, nc.gpsimd)
        _load_w("wv", 0, nc.gpsimd)
        _load_w("wv", 1, nc.gpsimd)
        wo2 = []
        for j in range(HPC):
            wt_ = wopool.tile([128, C], bf16, tag=f"wo{j}", name=f"wo{j}")
            nc.sync.dma_start(wt_[:], wo[j * 128:(j + 1) * 128, :])
            wo2.append(wt_)
        mb_t = cstp.tile([128, 128], bf16, tag="mb")
        nc.gpsimd.dma_start(mb_t[:], mbd)
        onesk = cstp.tile([128, 1], bf16, tag="onesk")
        nc.gpsimd.dma_start(onesk[:], onesk_d)
        ones1 = cstp.tile([1, 128], f32r, tag="ones1")
        nc.gpsimd.dma_start(ones1[:], ones1_d)

        # resident projection outputs (bf16)
        qT = [resq.tile([128, T], bf16, tag=f"qT{h}", name=f"qT{h}")
              for h in range(HPC)]
        kT = [resq.tile([128, T], bf16, tag=f"kT{h}", name=f"kT{h}")
              for h in range(HPC)]
        vres = [resq.tile([128, DH], bf16, tag=f"v{kb}", name=f"v{kb}")
                for kb in range(T // 128)]

        # ------------------------------------------------------------------
        # stage A: projections + RoPE into resident tiles
        # ------------------------------------------------------------------
        with ExitStack() as phA:
            ps1 = phA.enter_context(tc.tile_pool(name="ps1", bufs=6, space="PSUM"))

            for tci in range(NT):
                tsl = bass.ts(tci, TCH)
                xt = []
                for ci in range(NCH):
                    t_ = xpool.tile([128, TCH], bf16, tag=f"x{ci}")
                    nc.scalar.dma_start(t_[:], xT[ci * 128:(ci + 1) * 128, tsl])
                    xt.append(t_)

                # Q^T and K^T with RoPE
                for wname, dest in (("wq", qT), ("wk", kT)):
                    for h in range(HPC):
                        ps = ps1.tile([128, TCH], f32, tag="ps1")
                        for ci in range(NCH):
                            nc.tensor.matmul(
                                ps[:],
                                w_tiles[(wname, ci)][:, h * 128:(h + 1) * 128],
                                xt[ci][:],
                                start=(ci == 0),
                                stop=(ci == NCH - 1),
                            )
                        c_sl = cos_t[:, tsl]
                        s_sl = sin_t[:, tsl]
                        out_t = dest[h]
                        t1 = rt.tile([64, TCH], f32, tag="r1")
                        nc.vector.tensor_mul(t1[:], ps[0:64, :], c_sl)
                        t2 = rt.tile([64, TCH], f32, tag="r2")
                        nc.vector.tensor_mul(t2[:], ps[64:128, :], s_sl)
                        nc.vector.tensor_sub(out_t[0:64, tsl], t1[:], t2[:])
                        t3 = rt.tile([64, TCH], f32, tag="r3")
                        nc.vector.tensor_mul(t3[:], ps[0:64, :], s_sl)
                        t4 = rt.tile([64, TCH], f32, tag="r4")
                        nc.vector.tensor_mul(t4[:], ps[64:128, :], c_sl)
                        nc.vector.tensor_add(out_t[64:128, tsl], t3[:], t4[:])

                # V (natural [t, d] orientation)
                for tsi in range(TCH // 128):
                    ps = ps1.tile([128, DH], f32, tag="ps1")
                    for ci in range(NCH):
                        nc.tensor.matmul(
                            ps[:],
                            xt[ci][:, tsi * 128:(tsi + 1) * 128],
                            w_tiles[("wv", ci)][:],
                            start=(ci == 0),
                            stop=(ci == NCH - 1),
                        )
                    kb = tci * (TCH // 128) + tsi
                    nc.vector.tensor_copy(vres[kb][:], ps[:])

        # ------------------------------------------------------------------
        # stage B/C: attention (qc outer, head inner) + interleaved WO
        # ------------------------------------------------------------------
        pss = top.enter_context(tc.tile_pool(name="pss", bufs=3, space="PSUM"))
        pso = top.enter_context(tc.tile_pool(name="pso", bufs=2, space="PSUM"))
        psr = top.enter_context(tc.tile_pool(name="psr", bufs=2, space="PSUM"))
        psb = top.enter_context(tc.tile_pool(name="psb", bufs=1, space="PSUM"))

        ot_tiles = {}
        pending = [None]

        def _emit_norm(h, qc, pso_t, psr_t, piecewise=False):
            # normalize: copy rowsums to SBUF, broadcast via 1-wide
            # matmul, reciprocal + multiply on DVE; deferred a couple of
            # pipeline steps so the PE never waits on the rsc copy
            # latency.  piecewise=True splits the reciprocal/multiply
            # into per-128-column pieces so a WO chain emitted right
            # after can start on piece 0 instead of waiting ~4us for
            # the full-width reciprocal (used for the last chunk).
            rsc = rp.tile([1, QCH], f32r, tag="rsc")
            nc.scalar.copy(rsc[:], psr_t[:])
            psb_t = psb.tile([128, QCH], f32, tag="b")
            nc.tensor.matmul(psb_t[:], ones1[:], rsc[:], start=True, stop=True)
            bsb_t = bp.tile([128, QCH], f32, tag="bsb")
            ot = otp.tile([128, QCH], bf16, tag=f"ot{h}", name=f"ot{h}")
            if piecewise:
                for qs in range(QCH // 128):
                    sl = slice(qs * 128, (qs + 1) * 128)
                    nc.vector.reciprocal(bsb_t[:, sl], psb_t[:, sl])
                    nc.vector.tensor_mul(ot[:, sl], pso_t[:, sl], bsb_t[:, sl])
            else:
                nc.vector.reciprocal(bsb_t[:], psb_t[:])
                nc.vector.tensor_mul(ot[:], pso_t[:], bsb_t[:])
            ot_tiles[(h, qc)] = ot

        _ydma_engs = [nc.sync, nc.scalar, nc.gpsimd]

        def _emit_wo(qc):
            for qs in range(QCH // 128):
                row0 = qc * QCH + qs * 128
                for cc in range(C // 512):
                    psy_t = pss.tile([128, 512], f32, tag="s")
                    for hh in range(HPC):
                        nc.tensor.matmul(
                            psy_t[:],
                            ot_tiles[(hh, qc)][:, qs * 128:(qs + 1) * 128],
                            wo2[hh][:, cc * 512:(cc + 1) * 512],
                            start=(hh == 0),
                            stop=(hh == HPC - 1),
                        )
                    ysb_t = ysbp.tile([128, 512], f32, tag="ysb")
                    idx = qs * 4 + cc
                    if idx % 2 == 0:
                        nc.scalar.copy(ysb_t[:], psy_t[:])
                    else:
                        nc.vector.tensor_copy(ysb_t[:], psy_t[:])
                    _ydma_engs[idx % 3].dma_start(
                        y[row0:row0 + 128, cc * 512:(cc + 1) * 512], ysb_t[:]
                    )

        # Software-pipelined attention: the AV/rowsum matmuls for block k
        # are emitted only after the S matmuls for blocks k+1 and k+2, so
        # by the time the PE reaches AV(k) the ACT exp(k) it depends on
        # finished long ago.  The pipeline is carried across (head,
        # q-chunk) boundaries, which also removes the drain bubbles at
        # chunk transitions.  pss bufs=3 covers the three in-flight S
        # tiles; pso/psr bufs=2 cover the current + previous chunk.
        pipe = []

        def _pop_block():
            ent = pipe.pop(0)
            kb, e, qlo, pso_t, psr_t, kmax, h, qc = ent
            nc.tensor.matmul(
                pso_t[:, qlo:],
                vres[kb][:, h * 128:(h + 1) * 128],
                e[:, qlo:],
                start=(kb == 0),
                stop=(kb == kmax),
            )
            nc.tensor.matmul(
                psr_t[:, qlo:], onesk[:], e[:, qlo:],
                start=(kb == 0), stop=(kb == kmax),
            )
            if kb == kmax:
                pending[0] = (h, qc, pso_t, psr_t)
            elif pending[0] is not None:
                _emit_norm(*pending[0])
                pending[0] = None

        def _attention(qc, h):
            kmax = 4 * qc + 3
            q_sl = qT[h][:, qc * QCH:(qc + 1) * QCH]
            pso_t = pso.tile([128, QCH], f32, tag="o")
            psr_t = psr.tile([1, QCH], f32, tag="rs")
            for kb in range(kmax + 1):
                # diagonal blocks: columns q < 128*i_rel are fully
                # causally masked -- skip them entirely; only the
                # [128,128] square at the diagonal needs a mask
                i_rel = kb - 4 * qc
                qlo = 128 * i_rel if i_rel > 0 else 0
                pss_t = pss.tile([128, QCH], f32, tag="s")
                nc.tensor.matmul(
                    pss_t[:, qlo:],
                    kT[h][:, kb * 128:(kb + 1) * 128],
                    q_sl[:, qlo:],
                    start=True,
                    stop=True,
                )
                e = ep.tile([128, QCH], bf16, tag="e")
                nc.scalar.activation(
                    e[:, qlo:], pss_t[:, qlo:], AF.Exp, scale=SCALE
                )
                if i_rel >= 0:  # triangle mask on the diagonal square
                    nc.gpsimd.tensor_mul(
                        e[:, qlo:qlo + 128], e[:, qlo:qlo + 128], mb_t[:]
                    )
                pipe.append((kb, e, qlo, pso_t, psr_t, kmax, h, qc))
                if len(pipe) > 2:
                    _pop_block()

        for qc in range(NQ):
            for h in range(HPC):
                _attention(qc, h)
                if h == 0 and qc > 0:
                    # previous q-chunk's 4 heads are all normalized by
                    # now; emit its output projection here so the y
                    # DMAs spread across the attention window
                    _emit_wo(qc - 1)
        while pipe:
            _pop_block()
        if pending[0] is not None:
            _emit_norm(*pending[0], piecewise=True)
            pending[0] = None
        _emit_wo(NQ - 1)

    _split_waits(nc)
    return nc


_CACHED_NC = None


def _get_nc():
    global _CACHED_NC
    if _CACHED_NC is None:
        _CACHED_NC = _build_nc()
    return _CACHED_NC


# --------------------------------------------------------------------------
# host-side input prep / gather
# --------------------------------------------------------------------------
def _deinterleave_perm():
    """per-head column permutation: [2j for j<64] then [2j+1]"""
    p = np.empty(D, dtype=np.int64)
    p[:64] = np.arange(0, D, 2)
    p[64:] = np.arange(1, D, 2)
    return p


def _bf16(a):
    import ml_dtypes

    return np.ascontiguousarray(a).astype(ml_dtypes.bfloat16)


def _make_core_inputs(x, freqs_cos, freqs_sin, wq, wk, wv, wo):
    x = np.asarray(x, dtype=np.float32)
    freqs_cos = np.asarray(freqs_cos, dtype=np.float32)
    freqs_sin = np.asarray(freqs_sin, dtype=np.float32)
    wq = np.asarray(wq, dtype=np.float32)
    wk = np.asarray(wk, dtype=np.float32)
    wv = np.asarray(wv, dtype=np.float32)
    wo = np.asarray(wo, dtype=np.float32)

    perm = _deinterleave_perm()
    cosT = np.ascontiguousarray(freqs_cos.T)  # [64, T]
    sinT = np.ascontiguousarray(freqs_sin.T)

    # causal staircase: mb[k, j] = 1 iff k <= j
    k_idx = np.arange(128)[:, None]
    j_idx = np.arange(128)[None, :]
    mb = _bf16((k_idx <= j_idx).astype(np.float32))

    onesk = _bf16(np.ones((128, 1), dtype=np.float32))
    ones1 = np.ones((1, 128), dtype=np.float32)

    xTb = [_bf16(x[b].T) for b in range(B)]

    in_maps = []
    for core in range(N_CORES):
        b, hg = core // 4, core % 4
        cols = slice(hg * DH, (hg + 1) * DH)
        wq_s = wq[:, cols].reshape(C, HPC, D)[:, :, perm].reshape(C, DH)
        wk_s = wk[:, cols].reshape(C, HPC, D)[:, :, perm].reshape(C, DH)
        in_maps.append({
            "xT": xTb[b],
            "wq": _bf16(wq_s),
            "wk": _bf16(wk_s),
            "wv": _bf16(wv[:, cols]),
            "wo": _bf16(wo[cols, :]),
            "cosT": cosT,
            "sinT": sinT,
            "mb": mb,
            "onesk": onesk,
            "ones1": ones1,
        })
    return in_maps


def kernel(x, freqs_cos, freqs_sin, wq, wk, wv, wo, _trace=False, _trace_kwargs=None):
    nc = _get_nc()
    in_maps = _make_core_inputs(x, freqs_cos, freqs_sin, wq, wk, wv, wo)
    res = run_bass_kernel_spmd(
        nc, in_maps, core_ids=list(range(N_CORES)), trace=_trace,
        **(_trace_kwargs or {}),
    )
    out = np.zeros((B, T, C), dtype=np.float32)
    for core in range(N_CORES):
        out[core // 4] += res.results[core]["y"]
    if _trace:
        kernel.last_results = res
    return out


# revision 11
# speedup vs baseline: 1.1885x; 1.1885x over previous
"""Multi-head attention (B=2, T=2048, C=2048, H=16, causal, rotary) on 8
Trainium2 NeuronCores.

Sharding: tensor-parallel over heads x data-parallel over batch.
Core c handles batch b = c // 4 and heads [4*(c%4), 4*(c%4)+4).
Each core computes a partial output y_c = attn_out(4 heads) @ wo_rows;
the host sums the 4 partials per batch (row-parallel wo).

Fused single-phase design: Q^T/K^T/V are computed once and stay
SBUF-resident in bf16 (no DRAM spill round-trip).  All matmuls run in
bf16 (1 cycle/row, same PE rate as f32r); PSUM accumulation is fp32.

  stage A (per t-chunk 512): Q^T,K^T [d,t] per head via stationary-
          weight matmuls over host-pre-transposed x^T; RoPE applied in
          de-interleaved form (host permutes wq/wk columns so rows
          0:64 = real pairs, 64:128 = imag pairs) with stacked cos|sin
          tiles (2 muls + sub + add on DVE); V in natural [t,d] via
          gpsimd copies.  Results land in resident bf16 tiles.
  stage B (per q-chunk 512, head): S^T[k,q] tiles by one matmul each
          (contraction d=128), ACT exp with 1/sqrt(D) folded in, causal
          via block skip + staircase mask multiply on gpsimd, O^T
          accumulated with V stationary / E moving, row-sums via
          ones-column matmul, reciprocal broadcast via 1-wide matmul,
          normalize on DVE (deferred one iteration so the PE never
          waits on the scalar-engine copy latency).
  stage C: wo applied per q-chunk as soon as its 4 heads are
          normalized, with per-[128,512] y pieces DMA'd out on the
          sync queue so the output stream overlaps attention instead
          of draining at the end.
"""

import math
import os
import sys
from contextlib import ExitStack

import numpy as np

for _p in ("/opt/trn_rl_repo", "/root/.axon_site/_ro/trn_rl_repo"):
    if os.path.isdir(_p) and _p not in sys.path:
        sys.path.append(_p)

import bass_rust
import concourse.bass as bass
import concourse.mybir as mybir
import concourse.tile as tile
from concourse.bass_utils import run_bass_kernel_spmd
from concourse.vector_clock import ScopedClock, VectorClock

B, T, C, H = 2, 2048, 2048, 16
D = 128
HPC = H // 4          # 4 heads per core
DH = HPC * D          # 512 head-dims per core
NCH = C // 128        # 16 contraction chunks
TCH = 512             # stage-A t-chunk
QCH = 512             # stage-B q-chunk
NT = T // TCH
NQ = T // QCH
N_CORES = 8
SCALE = 1.0 / math.sqrt(D)

f32 = mybir.dt.float32
f32r = mybir.dt.float32r
bf16 = mybir.dt.bfloat16
AF = mybir.ActivationFunctionType


# --------------------------------------------------------------------------
# toolchain workarounds
# --------------------------------------------------------------------------
def _patched_drain_and_barrier(self, tick_clock, wait_clock):
    """walrus codegen accepts only one sem wait on an InstDrain; emit one
    drain per outstanding proc instead of one drain with N waits."""
    ticks = list(tick_clock.global_clock)
    for i, t in enumerate(ticks):
        if t <= 0:
            continue
        sub = VectorClock([t if j == i else 0 for j in range(len(ticks))])
        d = self.nc.sync.drain()
        wait_clock.add_sem_waits(d.ins, ScopedClock({None: sub}))
    self.nc.all_engine_barrier()
    assert self.sems is not None
    popped = self.nc._tile_sem_poison_stack.pop()
    assert popped is self._sem_poison
    self.nc.clear_and_free_semaphores(list(self.sems.allocated().values()))
    self.nc.all_engine_barrier()


tile.TileContext._drain_and_barrier = _patched_drain_and_barrier

_SPLIT_OPS = {
    "Matmult", "Drain", "DMACopy", "DMATransposeAnt", "Activation", "TensorTensor", "TensorReduce",
    "TensorCopy", "Reciprocal", "TensorScalarPtr", "TensorScalar",
    "CopyPredicated", "Memset", "NoOp", "Pool", "Max", "MaxIndex",
    "StreamShuffle", "StreamTranspose", "TensorTensorScan",
    "ScalarTensorTensor", "TensorTensorReduce", "Iota", "BNStats",
    "BNStatsAggregate", "Select",
}
_ws_counter = [0]


def _split_waits(nc, limit=1):
    """walrus encodes a limited number of sem waits on engine instructions
    (fused LDW+MM and Drain take only one). Move excess waits onto
    same-engine NoOps inserted immediately before; engine program order
    preserves semantics."""
    for f in nc.m.functions:
        for b in f.blocks:
            insts = b.instructions
            i = 0
            while i < len(insts):
                inst = insts[i]
                si = inst.sync_info
                if (
                    inst.opcode not in _SPLIT_OPS
                    or si is None
                    or not si.on_wait
                    or len(si.on_wait) <= limit
                ):
                    i += 1
                    continue
                waits = list(si.on_wait)
                extra, keep = waits[:-limit], waits[-limit:]
                for w in extra:
                    _ws_counter[0] += 1
                    nop = bass_rust.InstNoOp(
                        name=f"I-waitsplit-{_ws_counter[0]}", engine=inst.engine
                    )
                    nop.sync_info = mybir.SyncInfo(on_wait=[w], on_update=[])
                    insts.insert(i, nop)
                    i += 1
                inst.sync_info = mybir.SyncInfo(
                    on_wait=keep,
                    on_update=list(si.on_update) if si.on_update else [],
                )
                i += 1


# --------------------------------------------------------------------------
# kernel build
# --------------------------------------------------------------------------
def _build_nc():
    nc = bass.Bass("TRN2", debug=False, target_bir_lowering=False)

    xT = nc.dram_tensor("xT", [C, T], bf16, kind="ExternalInput").ap()
    wq = nc.dram_tensor("wq", [C, DH], bf16, kind="ExternalInput").ap()
    wk = nc.dram_tensor("wk", [C, DH], bf16, kind="ExternalInput").ap()
    wv = nc.dram_tensor("wv", [C, DH], bf16, kind="ExternalInput").ap()
    wo = nc.dram_tensor("wo", [DH, C], bf16, kind="ExternalInput").ap()
    cosT = nc.dram_tensor("cosT", [64, T], f32, kind="ExternalInput").ap()
    sinT = nc.dram_tensor("sinT", [64, T], f32, kind="ExternalInput").ap()
    mbd = nc.dram_tensor("mb", [128, 128], bf16, kind="ExternalInput").ap()
    onesk_d = nc.dram_tensor("onesk", [128, 1], bf16, kind="ExternalInput").ap()
    ones1_d = nc.dram_tensor("ones1", [1, 128], f32r, kind="ExternalInput").ap()
    y = nc.dram_tensor("y", [T, C], f32, kind="ExternalOutput").ap()

    with tile.TileContext(nc) as tc, ExitStack() as top:
        # resident pools (live for the whole kernel)
        wpool = top.enter_context(tc.tile_pool(name="w", bufs=1))
        wopool = top.enter_context(tc.tile_pool(name="wo", bufs=1))
        cspool = top.enter_context(tc.tile_pool(name="cs", bufs=1))
        cstp = top.enter_context(tc.tile_pool(name="cst", bufs=1))
        xpool = top.enter_context(tc.tile_pool(name="x", bufs=2))
        resq = top.enter_context(tc.tile_pool(name="qkv", bufs=1))
        rt = top.enter_context(tc.tile_pool(name="rt", bufs=2))
        ep = top.enter_context(tc.tile_pool(name="e", bufs=4))
        otp = top.enter_context(tc.tile_pool(name="ot", bufs=2))
        rp = top.enter_context(tc.tile_pool(name="r", bufs=2))
        bp = top.enter_context(tc.tile_pool(name="bsb", bufs=2))
        ysbp = top.enter_context(tc.tile_pool(name="ysb", bufs=4))

        # ---- weight/constant loads, split across queues so the first
        # matmul's operands land ~600ns after the preamble ----
        w_tiles = {}
        for ci in range(NCH):
            wt = wpool.tile([128, DH], bf16, tag=f"wq{ci}", name=f"wq{ci}")
            eng = nc.sync if ci % 2 == 0 else nc.gpsimd
            eng.dma_start(wt[:], wq[ci * 128:(ci + 1) * 128, :])
            w_tiles[("wq", ci)] = wt
        for ci in range(NCH):
            wt = wpool.tile([128, DH], bf16, tag=f"wk{ci}", name=f"wk{ci}")
            nc.sync.dma_start(wt[:], wk[ci * 128:(ci + 1) * 128, :])
            w_tiles[("wk", ci)] = wt
        for ci in range(NCH):
            wt = wpool.tile([128, DH], bf16, tag=f"wv{ci}", name=f"wv{ci}")
            nc.gpsimd.dma_start(wt[:], wv[ci * 128:(ci + 1) * 128, :])
            w_tiles[("wv", ci)] = wt

        cos_t = cspool.tile([64, T], f32, tag="cos")
        nc.gpsimd.dma_start(cos_t[:], cosT)
        sin_t = cspool.tile([64, T], f32, tag="sin")
        nc.gpsimd.dma_start(sin_t[:], sinT)
        mb_t = cstp.tile([128, 128], bf16, tag="mb")
        nc.gpsimd.dma_start(mb_t[:], mbd)
        onesk = cstp.tile([128, 1], bf16, tag="onesk")
        nc.gpsimd.dma_start(onesk[:], onesk_d)
        ones1 = cstp.tile([1, 128], f32r, tag="ones1")
        nc.gpsimd.dma_start(ones1[:], ones1_d)

        wo2 = []
        for j in range(HPC):
            wt_ = wopool.tile([128, C], bf16, tag=f"wo{j}", name=f"wo{j}")
            nc.sync.dma_start(wt_[:], wo[j * 128:(j + 1) * 128, :])
            wo2.append(wt_)

        # resident projection outputs (bf16)
        qT = [resq.tile([128, T], bf16, tag=f"qT{h}", name=f"qT{h}")
              for h in range(HPC)]
        kT = [resq.tile([128, T], bf16, tag=f"kT{h}", name=f"kT{h}")
              for h in range(HPC)]
        vres = [resq.tile([128, DH], bf16, tag=f"v{kb}", name=f"v{kb}")
                for kb in range(T // 128)]

        # ------------------------------------------------------------------
        # stage A: projections + RoPE into resident tiles
        # ------------------------------------------------------------------
        with ExitStack() as phA:
            ps1 = phA.enter_context(tc.tile_pool(name="ps1", bufs=6, space="PSUM"))

            for tci in range(NT):
                tsl = bass.ts(tci, TCH)
                xt = []
                for ci in range(NCH):
                    t_ = xpool.tile([128, TCH], bf16, tag=f"x{ci}")
                    nc.scalar.dma_start(t_[:], xT[ci * 128:(ci + 1) * 128, tsl])
                    xt.append(t_)

                # Q^T and K^T with RoPE
                for wname, dest in (("wq", qT), ("wk", kT)):
                    for h in range(HPC):
                        ps = ps1.tile([128, TCH], f32, tag="ps1")
                        for ci in range(NCH):
                            nc.tensor.matmul(
                                ps[:],
                                w_tiles[(wname, ci)][:, h * 128:(h + 1) * 128],
                                xt[ci][:],
                                start=(ci == 0),
                                stop=(ci == NCH - 1),
                            )
                        c_sl = cos_t[:, tsl]
                        s_sl = sin_t[:, tsl]
                        out_t = dest[h]
                        t1 = rt.tile([64, TCH], f32, tag="r1")
                        nc.vector.tensor_mul(t1[:], ps[0:64, :], c_sl)
                        t2 = rt.tile([64, TCH], f32, tag="r2")
                        nc.vector.tensor_mul(t2[:], ps[64:128, :], s_sl)
                        nc.vector.tensor_sub(out_t[0:64, tsl], t1[:], t2[:])
                        t3 = rt.tile([64, TCH], f32, tag="r3")
                        nc.vector.tensor_mul(t3[:], ps[0:64, :], s_sl)
                        t4 = rt.tile([64, TCH], f32, tag="r4")
                        nc.vector.tensor_mul(t4[:], ps[64:128, :], c_sl)
                        nc.vector.tensor_add(out_t[64:128, tsl], t3[:], t4[:])

                # V (natural [t, d] orientation)
                for tsi in range(TCH // 128):
                    ps = ps1.tile([128, DH], f32, tag="ps1")
                    for ci in range(NCH):
                        nc.tensor.matmul(
                            ps[:],
                            xt[ci][:, tsi * 128:(tsi + 1) * 128],
                            w_tiles[("wv", ci)][:],
                            start=(ci == 0),
                            stop=(ci == NCH - 1),
                        )
                    kb = tci * (TCH // 128) + tsi
                    nc.vector.tensor_copy(vres[kb][:], ps[:])

        # ------------------------------------------------------------------
        # stage B/C: attention (qc outer, head inner) + interleaved WO
        # ------------------------------------------------------------------
        pss = top.enter_context(tc.tile_pool(name="pss", bufs=3, space="PSUM"))
        pso = top.enter_context(tc.tile_pool(name="pso", bufs=2, space="PSUM"))
        psr = top.enter_context(tc.tile_pool(name="psr", bufs=2, space="PSUM"))
        psb = top.enter_context(tc.tile_pool(name="psb", bufs=1, space="PSUM"))

        ot_tiles = {}
        pending = [None]

        def _emit_norm(h, qc, pso_t, psr_t, piecewise=False):
            # normalize: copy rowsums to SBUF, broadcast via 1-wide
            # matmul, reciprocal + multiply on DVE; deferred one (h,qc)
            # iteration so the PE never waits on the rsc copy latency.
            # piecewise=True splits the reciprocal/multiply into
            # per-128-column pieces so a WO chain emitted right after
            # can start on piece 0 instead of waiting ~4us for the
            # full-width reciprocal (used for the last chunk).
            rsc = rp.tile([1, QCH], f32r, tag="rsc")
            nc.scalar.copy(rsc[:], psr_t[:])
            psb_t = psb.tile([128, QCH], f32, tag="b")
            nc.tensor.matmul(psb_t[:], ones1[:], rsc[:], start=True, stop=True)
            bsb_t = bp.tile([128, QCH], f32, tag="bsb")
            ot = otp.tile([128, QCH], bf16, tag=f"ot{h}", name=f"ot{h}")
            if piecewise:
                for qs in range(QCH // 128):
                    sl = slice(qs * 128, (qs + 1) * 128)
                    nc.vector.reciprocal(bsb_t[:, sl], psb_t[:, sl])
                    nc.vector.tensor_mul(ot[:, sl], pso_t[:, sl], bsb_t[:, sl])
            else:
                nc.vector.reciprocal(bsb_t[:], psb_t[:])
                nc.vector.tensor_mul(ot[:], pso_t[:], bsb_t[:])
            ot_tiles[(h, qc)] = ot

        def _emit_wo(qc):
            for qs in range(QCH // 128):
                row0 = qc * QCH + qs * 128
                for cc in range(C // 512):
                    psy_t = pss.tile([128, 512], f32, tag="s")
                    for hh in range(HPC):
                        nc.tensor.matmul(
                            psy_t[:],
                            ot_tiles[(hh, qc)][:, qs * 128:(qs + 1) * 128],
                            wo2[hh][:, cc * 512:(cc + 1) * 512],
                            start=(hh == 0),
                            stop=(hh == HPC - 1),
                        )
                    ysb_t = ysbp.tile([128, 512], f32, tag="ysb")
                    nc.scalar.copy(ysb_t[:], psy_t[:])
                    nc.sync.dma_start(
                        y[row0:row0 + 128, cc * 512:(cc + 1) * 512], ysb_t[:]
                    )

        def _attention(qc, h):
            kmax = 4 * qc + 3
            q_sl = qT[h][:, qc * QCH:(qc + 1) * QCH]
            pso_t = pso.tile([128, QCH], f32, tag="o")
            psr_t = psr.tile([1, QCH], f32, tag="rs")
            first_kb_done = False
            for kb in range(kmax + 1):
                # diagonal blocks: columns q < 128*i_rel are fully
                # causally masked -- skip them entirely; only the
                # [128,128] square at the diagonal needs a mask
                i_rel = kb - 4 * qc
                qlo = 128 * i_rel if i_rel > 0 else 0
                pss_t = pss.tile([128, QCH], f32, tag="s")
                nc.tensor.matmul(
                    pss_t[:, qlo:],
                    kT[h][:, kb * 128:(kb + 1) * 128],
                    q_sl[:, qlo:],
                    start=True,
                    stop=True,
                )
                e = ep.tile([128, QCH], bf16, tag="e")
                nc.scalar.activation(
                    e[:, qlo:], pss_t[:, qlo:], AF.Exp, scale=SCALE
                )
                if i_rel >= 0:  # triangle mask on the diagonal square
                    nc.gpsimd.tensor_mul(
                        e[:, qlo:qlo + 128], e[:, qlo:qlo + 128], mb_t[:]
                    )
                nc.tensor.matmul(
                    pso_t[:, qlo:],
                    vres[kb][:, h * 128:(h + 1) * 128],
                    e[:, qlo:],
                    start=(kb == 0),
                    stop=(kb == kmax),
                )
                nc.tensor.matmul(
                    psr_t[:, qlo:], onesk[:], e[:, qlo:],
                    start=(kb == 0), stop=(kb == kmax),
                )
                if not first_kb_done:
                    first_kb_done = True
                    if pending[0] is not None:
                        _emit_norm(*pending[0])
                        pending[0] = None
            pending[0] = (h, qc, pso_t, psr_t)

        for qc in range(NQ):
            for h in range(HPC):
                _attention(qc, h)
                if h == 0 and qc > 0:
                    # previous q-chunk's 4 heads are all normalized by
                    # now; emit its output projection here so the y
                    # DMAs spread across the attention window
                    _emit_wo(qc - 1)
        if pending[0] is not None:
            _emit_norm(*pending[0], piecewise=True)
            pending[0] = None
        _emit_wo(NQ - 1)

    _split_waits(nc)
    return nc


_CACHED_NC = None


def _get_nc():
    global _CACHED_NC
    if _CACHED_NC is None:
        _CACHED_NC = _build_nc()
    return _CACHED_NC


# --------------------------------------------------------------------------
# host-side input prep / gather
# --------------------------------------------------------------------------
def _deinterleave_perm():
    """per-head column permutation: [2j for j<64] then [2j+1]"""
    p = np.empty(D, dtype=np.int64)
    p[:64] = np.arange(0, D, 2)
    p[64:] = np.arange(1, D, 2)
    return p


def _bf16(a):
    import ml_dtypes

    return np.ascontiguousarray(a).astype(ml_dtypes.bfloat16)


def _make_core_inputs(x, freqs_cos, freqs_sin, wq, wk, wv, wo):
    x = np.asarray(x, dtype=np.float32)
    freqs_cos = np.asarray(freqs_cos, dtype=np.float32)
    freqs_sin = np.asarray(freqs_sin, dtype=np.float32)
    wq = np.asarray(wq, dtype=np.float32)
    wk = np.asarray(wk, dtype=np.float32)
    wv = np.asarray(wv, dtype=np.float32)
    wo = np.asarray(wo, dtype=np.float32)

    perm = _deinterleave_perm()
    cosT = np.ascontiguousarray(freqs_cos.T)  # [64, T]
    sinT = np.ascontiguousarray(freqs_sin.T)

    # causal staircase: mb[k, j] = 1 iff k <= j
    k_idx = np.arange(128)[:, None]
    j_idx = np.arange(128)[None, :]
    mb = _bf16((k_idx <= j_idx).astype(np.float32))

    onesk = _bf16(np.ones((128, 1), dtype=np.float32))
    ones1 = np.ones((1, 128), dtype=np.float32)

    xTb = [_bf16(x[b].T) for b in range(B)]

    in_maps = []
    for core in range(N_CORES):
        b, hg = core // 4, core % 4
        cols = slice(hg * DH, (hg + 1) * DH)
        wq_s = wq[:, cols].reshape(C, HPC, D)[:, :, perm].reshape(C, DH)
        wk_s = wk[:, cols].reshape(C, HPC, D)[:, :, perm].reshape(C, DH)
        in_maps.append({
            "xT": xTb[b],
            "wq": _bf16(wq_s),
            "wk": _bf16(wk_s),
            "wv": _bf16(wv[:, cols]),
            "wo": _bf16(wo[cols, :]),
            "cosT": cosT,
            "sinT": sinT,
            "mb": mb,
            "onesk": onesk,
            "ones1": ones1,
        })
    return in_maps


def kernel(x, freqs_cos, freqs_sin, wq, wk, wv, wo, _trace=False, _trace_kwargs=None):
    nc = _get_nc()
    in_maps = _make_core_inputs(x, freqs_cos, freqs_sin, wq, wk, wv, wo)
    res = run_bass_kernel_spmd(
        nc, in_maps, core_ids=list(range(N_CORES)), trace=_trace,
        **(_trace_kwargs or {}),
    )
    out = np.zeros((B, T, C), dtype=np.float32)
    for core in range(N_CORES):
        out[core // 4] += res.results[core]["y"]
    if _trace:
        kernel.last_results = res
    return out
